# revision 1
# baseline (speedup 1.0000x reference)
"""Trainium2 Bass kernel: GQA attention layer (RoPE + causal sliding-window)
tensor-parallel across heads on 8 NeuronCores.

Problem shapes (hardcoded): S=2048 tokens, DIM=4096, HQ=32 q-heads,
HKV=8 kv-heads, HD=128 head dim, window=2048 (window >= S, so the mask is
plain causal).

Sharding: core c owns kv-head c and q-heads 4c..4c+3 (column-parallel
wq/wk/wv, row-parallel wo). Each core computes a full [S, DIM] partial of
the output projection in f16; the host sums the 8 partials in f32.

Device-side layout notes:
 - All matmul operands are bf16 (fp32 accumulate in PSUM).
 - Projections are computed in "transposed" layout qT/kT [HD, S] directly
   (out = W^T.T @ x^T), which is what the scoresT QK matmul wants. v gets a
   DMA-transpose back to natural [S, HD].
 - The head dim of q/k is de-interleaved (even dims in partitions 0..63,
   odd in 64..127) by permuting wq/wk columns on the host. RoPE is then two
   ACT partition-swap copies + four partition-aligned DVE ops per
   [128, 512] block. Dot products are permutation-invariant, so scores are
   unchanged.
 - scoresT blocks are [kj, qi]: adjacent kj tiles are paired into one
   2-bank PSUM tile so a single ACT exp processes [128, 1024] (amortizes
   the ~350-cycle ACT op overhead; ACT no longer paces the PE in phase B).
   Diagonal blocks are computed full-width (their upper-triangle part is
   valid-but-masked scores) and causal masking is gpsimd.affine_select
   after exp; PV/denominator matmuls still skip fully-masked columns.
   Softmax denominator = ones-matmul chain (stays on PE; PE has the
   headroom there vs ACT/DVE). No max subtraction: |scores*scale| < ~7 for
   these inputs, well within fp32/exp range.
 - DMA queue plan: inputs ride sync (x, half of wq) and gpsimd
   (wk/wv/wq/wo); cos/sin + all output DMAs ride the scalar (ACT) HWDGE
   queue. In the steady-state timing loop the next iteration's input DMAs
   therefore overlap this iteration's phase-C output drain.
"""

from contextlib import ExitStack, nullcontext

import numpy as np
import ml_dtypes

import concourse.bass as bass
import concourse.mybir as mybir
import concourse.tile as tile
from concourse import bacc
from concourse.bass_utils import run_bass_kernel_spmd

S = 2048
DIM = 4096
HQ, HKV, HD = 32, 8, 128
NCORES = 8
GH = HQ // HKV          # q heads per core (= per kv head) = 4
P = 128
KT = DIM // P           # 32 contraction tiles
SC = 512                # s-chunk (psum free dim)
NSC = S // SC           # 4
NQT = S // P            # 16 query tiles of 128
NMC = DIM // SC         # 8 output column chunks
SCALE = float(HD) ** -0.5

F32 = mybir.dt.float32
F16 = mybir.dt.float16
BF16 = mybir.dt.bfloat16

_CACHE = {}


def _build_bass(loop_n=1, staggered=False, phases="ABC", dn_on_pe=False):
    """loop_n > 1 wraps the whole body in a hardware For_i loop — used only
    by the test harness for differential wall-clock timing (the axon
    dispatch floor is ~80 ms, far above the kernel's execution time).
    phases: subset of "ABC" — ablation probes only (correctness needs ABC)."""
    nc = bacc.Bacc("TRN2", target_bir_lowering=False, debug=False,
                   enable_asserts=False)
    xT_d = nc.dram_tensor("xt", [DIM, S], BF16, kind="ExternalInput")
    wq_d = nc.dram_tensor("wqt", [DIM, GH * HD], BF16, kind="ExternalInput")
    wk_d = nc.dram_tensor("wkt", [DIM, HD], BF16, kind="ExternalInput")
    wv_d = nc.dram_tensor("wvt", [DIM, HD], BF16, kind="ExternalInput")
    wo_d = nc.dram_tensor("wot", [GH * HD, DIM], BF16, kind="ExternalInput")
    cos_d = nc.dram_tensor("cos2", [P, S], BF16, kind="ExternalInput")
    sin_d = nc.dram_tensor("sin2", [P, S], BF16, kind="ExternalInput")
    out_d = nc.dram_tensor("out", [S, DIM], F16, kind="ExternalOutput")

    with tile.TileContext(nc) as tc, ExitStack() as ctx:
        consts = ctx.enter_context(tc.tile_pool(name="consts", bufs=1))
        state = ctx.enter_context(tc.tile_pool(name="state", bufs=1))
        xpool = ctx.enter_context(tc.tile_pool(name="xpool", bufs=2))
        ropep = ctx.enter_context(tc.tile_pool(name="ropep", bufs=2))
        expp = ctx.enter_context(tc.tile_pool(name="expp", bufs=6))
        osb = ctx.enter_context(tc.tile_pool(name="osb", bufs=2))
        rcp = ctx.enter_context(tc.tile_pool(name="rcp", bufs=2))
        vtp = ctx.enter_context(tc.tile_pool(name="vtp", bufs=2))
        # PSUM (8 banks): phase A projection chains and phase C wo chains
        # share mm_ps (2 banks); scores get 2x 2-bank tiles (paired exp);
        # pv+dn accumulators share 2 banks.
        mm_ps = ctx.enter_context(tc.tile_pool(name="mm_ps", bufs=2, space="PSUM"))
        sc_ps = ctx.enter_context(tc.tile_pool(name="sc_ps", bufs=2, space="PSUM"))
        acc_ps = ctx.enter_context(tc.tile_pool(name="acc_ps", bufs=2, space="PSUM"))

        # staggered_reset: stages (input+A01 | A23 | B | C) pipeline across
        # loop iterations — iteration i+1's input DMAs overlap iteration
        # i's attention/output phases instead of a full-barrier back edge.
        ALL_ENGINES = (mybir.EngineType.PE, mybir.EngineType.DVE,
                       mybir.EngineType.Activation, mybir.EngineType.Pool,
                       mybir.EngineType.SP)
        loop_cm = (tc.For_i(0, loop_n, 1, staggered_reset=staggered,
                            hint_engines=ALL_ENGINES)
                   if loop_n > 1 else nullcontext())
        loop_cm.__enter__()

        def stage_boundary():
            if loop_n > 1 and staggered:
                tc.stage_boundary()

        # ---- constants / weights in SBUF ----
        # The first x chunk gates all compute: split it across sync+gpsimd.
        # wk/wv go first on gpsimd (the k and v chains run before q chains),
        # wq follows split across both queues. cos/sin ride the scalar
        # queue (it is otherwise idle until phase C's output DMAs).
        # wk first on gpsimd (the k chain runs first and needs it);
        # xc0 round-robins over all three DMA-capable queues; wq splits
        # across sync+scalar so the q chains aren't gated on one queue.
        wk_sb = consts.tile([P, KT, HD], BF16)
        nc.gpsimd.dma_start(wk_sb[:], wk_d.ap().rearrange("(o p) m -> p o m", p=P))
        xc0 = xpool.tile([P, KT, SC], BF16, tag="x")
        x_engs = [nc.sync, nc.scalar, nc.gpsimd]
        for og in range(8):
            x_engs[og % 3].dma_start(
                xc0[:, 4 * og:4 * og + 4, :],
                xT_d.ap()[512 * og:512 * (og + 1), 0:SC]
                .rearrange("(o p) s -> p o s", p=P))
        wv_sb = consts.tile([P, KT, HD], BF16)
        nc.gpsimd.dma_start(wv_sb[:], wv_d.ap().rearrange("(o p) m -> p o m", p=P))
        cos_sb = consts.tile([P, S], BF16)
        nc.scalar.dma_start(cos_sb[:], cos_d.ap())
        sin_sb = consts.tile([P, S], BF16)
        nc.scalar.dma_start(sin_sb[:], sin_d.ap())
        wq_sb = consts.tile([P, KT, GH * HD], BF16)
        for og in range(8):
            eng = nc.sync if og % 2 == 1 else nc.scalar
            eng.dma_start(
                wq_sb[:, 4 * og:4 * og + 4, :],
                wq_d.ap()[512 * og:512 * (og + 1), :]
                .rearrange("(o p) m -> p o m", p=P))
        ones_sb = consts.tile([P, P], BF16)
        nc.vector.memset(ones_sb[:], 1.0)

        # state tiles
        qT_sb = state.tile([P, GH, S], BF16)     # rope'd q, permuted head dim
        kT_sb = state.tile([P, S], BF16)         # rope'd k, permuted head dim
        v_sb = state.tile([P, NQT, HD], BF16)    # v natural [s-tile, d]
        attnT_sb = state.tile([P, GH, S], BF16)  # attn out^T, standard head dim

        H = 64
        mul = mybir.AluOpType.mult
        CP = mybir.ActivationFunctionType.Copy

        def rope(ps, out_sl, sc):
            """ps: [128, 512] f32 psum, head dim de-interleaved (even dims
            at partitions 0..63, odd at 64..127). Writes bf16 out_sl."""
            cs = cos_sb[:, SC * sc:SC * (sc + 1)]
            sn = sin_sb[:, SC * sc:SC * (sc + 1)]
            Asw = ropep.tile([P, SC], F32, tag="ropeA")
            P1 = ropep.tile([P, SC], F32, tag="ropeB")
            # partition-swapped copy of ps (ACT can shift base partitions)
            nc.scalar.activation(Asw[0:H], ps[H:P], CP)
            nc.scalar.activation(Asw[H:P], ps[0:H], CP)
            nc.vector.tensor_tensor(P1[:], ps[:], cs, mul)        # e*c | o*c
            nc.vector.tensor_tensor(Asw[:], Asw[:], sn, mul)      # o*s | e*s
            nc.vector.tensor_tensor(out_sl[0:H], P1[0:H], Asw[0:H],
                                    mybir.AluOpType.subtract)
            nc.vector.tensor_tensor(out_sl[H:P], P1[H:P], Asw[H:P],
                                    mybir.AluOpType.add)

        # ---- phase A: QKV projections + RoPE, per s-chunk ----
        # Chain order k, v, q0..q3 so the first chain only needs xc + wk.
        for sc in range(NSC):
            if sc == 2:
                stage_boundary()
            if sc == 0:
                xc = xc0
            else:
                xc = xpool.tile([P, KT, SC], BF16, tag="x")
                for og in range(8):
                    nc.sync.dma_start(
                        xc[:, 4 * og:4 * og + 4, :],
                        xT_d.ap()[512 * og:512 * (og + 1),
                                  SC * sc:SC * (sc + 1)]
                        .rearrange("(o p) s -> p o s", p=P))
            ps = mm_ps.tile([P, SC], F32, tag="mm")
            for o in range(KT):
                nc.tensor.matmul(ps[:], wk_sb[:, o, :], xc[:, o, :],
                                 start=(o == 0), stop=(o == KT - 1))
            rope(ps, kT_sb[:, SC * sc:SC * (sc + 1)], sc)
            ps = mm_ps.tile([P, SC], F32, tag="mm")
            for o in range(KT):
                nc.tensor.matmul(ps[:], wv_sb[:, o, :], xc[:, o, :],
                                 start=(o == 0), stop=(o == KT - 1))
            vt = vtp.tile([P, SC], BF16, tag="vt")
            nc.scalar.activation(vt[:], ps[:], CP)
            for b in range(4):
                nc.sync.dma_start_transpose(v_sb[:, 4 * sc + b, :],
                                            vt[:, P * b:P * (b + 1)])
            for h in range(GH):
                ps = mm_ps.tile([P, SC], F32, tag="mm")
                for o in range(KT):
                    nc.tensor.matmul(ps[:], wq_sb[:, o, HD * h:HD * (h + 1)],
                                     xc[:, o, :], start=(o == 0),
                                     stop=(o == KT - 1))
                rope(ps, qT_sb[:, h, SC * sc:SC * (sc + 1)], sc)

        # wo weights ride in the xpool slots freed after the last x chunk
        # (gpsimd queue, after wq — needed only from phase C on)
        wo_sb = xpool.tile([P, GH, DIM], BF16, tag="x")
        for h in range(GH):
            nc.gpsimd.dma_start(wo_sb[:, h, :], wo_d.ap()[P * h:P * (h + 1), :])

        stage_boundary()

        if "B" not in phases:
            scr = nc.dram_tensor("scratch", [P, (GH + 2) * S], BF16,
                                 kind="Internal")
            nc.sync.dma_start(scr.ap()[:, 0:GH * S],
                              qT_sb[:].rearrange("p h s -> p (h s)"))
            nc.sync.dma_start(scr.ap()[:, GH * S:(GH + 1) * S], kT_sb[:])
            nc.sync.dma_start(scr.ap()[:, (GH + 1) * S:],
                              v_sb[:].rearrange("p t d -> p (t d)"))

        # ---- phase C work items (closures), used as PE fillers during B ----
        # C(qc) chains depend only on B(qc)'s attnT slices, so they slot
        # into B(qc+1)'s PE stream wherever it would otherwise stall on
        # ACT(exp)/Pool(select)/DVE(denominator) latency.
        def c_chain(qt, mc, st, tail=False):
            def go():
                if mc == 0:
                    st["ostg"] = osb.tile([P, DIM], F16, tag="ob",
                                          name=f"ostg_{qt}")
                # in the dense tail block acc_ps is free — alternate pools
                # so four banks rotate and the staging copies hide
                if tail and mc % 2 == 1:
                    wps = acc_ps.tile([P, SC], F32, tag="acc",
                                      name=f"wps_{qt}_{mc}")
                else:
                    wps = mm_ps.tile([P, SC], F32, tag="mm",
                                     name=f"wps_{qt}_{mc}")
                for h in range(GH):
                    nc.tensor.matmul(wps[:],
                                     attnT_sb[:, h, P * qt:P * (qt + 1)],
                                     wo_sb[:, h, SC * mc:SC * (mc + 1)],
                                     start=(h == 0), stop=(h == GH - 1))
                dst = st["ostg"][:, SC * mc:SC * (mc + 1)]
                if mc % 2 == 0:
                    nc.vector.tensor_copy(dst, wps[:])
                else:
                    nc.scalar.activation(dst, wps[:], CP)
                if mc == NMC - 1:
                    nc.sync.dma_start(out_d.ap()[P * qt:P * (qt + 1), :],
                                      st["ostg"][:])
            return go

        cwork = {qc: [] for qc in range(NSC)}
        if "C" in phases:
            for qt in range(NQT):
                st = {}
                for mc in range(NMC):
                    cwork[qt // 4].append((qt, mc, st))

        def pop_c(qc):
            """Emit one pending C chain from round qc-1 (if any)."""
            if qc >= 1 and cwork[qc - 1]:
                qt, mc, st = cwork[qc - 1].pop(0)
                c_chain(qt, mc, st)()

        # ---- phase B: attention per (query chunk, head) ----
        for qc in (range(NSC) if "B" in phases else ()):
            T = 4 * qc + 4        # causal: kj tiles 0..T-1 (always even)
            for h in range(GH):
                q_sl = qT_sb[:, h, SC * qc:SC * (qc + 1)]
                exs = []   # per kj tile: (ex_tile, sub-slot, column offset)
                pair_tiles = []
                dacc = None   # running denominator accumulator [P, 2, SC]
                add = mybir.AluOpType.add
                for j in range(T // 2):
                    if j >= 2 and j % 2 == 0:
                        # cover the exp-paced scores throttle with C work
                        pop_c(qc)
                    diag = (2 * j >= 4 * qc)
                    sps = sc_ps.tile([P, 2, SC], F32, tag="sc")
                    ex = expp.tile([P, 2, SC], BF16, tag="exp")
                    for s_ in range(2):
                        t = 2 * j + s_
                        # diagonal blocks: columns qi < 128*(t-4qc) are
                        # fully masked - compute only the suffix
                        off = max(0, P * (t - 4 * qc))
                        nc.tensor.matmul(sps[:, s_, off:],
                                         kT_sb[:, P * t:P * (t + 1)],
                                         q_sl[:, off:], start=True, stop=True)
                        exs.append((ex, s_, off))
                    if not diag:
                        # paired [128, 1024] exp amortizes the ACT op
                        # overhead (full tiles only — psum fully written)
                        nc.scalar.activation(ex[:], sps[:],
                                             mybir.ActivationFunctionType.Exp,
                                             scale=SCALE)
                    else:
                        for s_ in range(2):
                            t = 2 * j + s_
                            off = max(0, P * (t - 4 * qc))
                            nc.scalar.activation(ex[:, s_, off:],
                                                 sps[:, s_, off:],
                                                 mybir.ActivationFunctionType.Exp,
                                                 scale=SCALE)
                            if off > 0:
                                # fully-masked cols: exact zeros so the
                                # denominator accumulate below reads no
                                # uninitialized SBUF
                                nc.vector.memset(ex[:, s_, 0:off], 0.0)
                            # keep iff (y + off + 512*qc) - (128*t + x) >= 0
                            nc.gpsimd.affine_select(
                                out=ex[:, s_, off:], in_=ex[:, s_, off:],
                                compare_op=mybir.AluOpType.is_ge,
                                fill=0.0, base=SC * qc + off - P * t,
                                pattern=[[1, SC - off]], channel_multiplier=-1)
                    # running denominator accumulate (DVE), hidden behind
                    # the ACT exp stream (one [128, 1024] bf16 add per pair)
                    pair_tiles.append(ex)
                    if dn_on_pe:
                        pass
                    elif j == 1:
                        dacc = expp.tile([P, 2, SC], BF16, tag="dacc",
                                         bufs=1)
                        nc.vector.tensor_tensor(dacc[:], pair_tiles[0][:],
                                                pair_tiles[1][:], add)
                    elif j >= 2:
                        nc.vector.tensor_tensor(dacc[:], dacc[:], ex[:], add)
                # cover the last exp's latency before the PV chain needs it
                pop_c(qc)
                pv = acc_ps.tile([P, SC], F32, tag="acc")
                for t in range(T):
                    ex, s_, off = exs[t]
                    nc.tensor.matmul(pv[:, off:], v_sb[:, t, :],
                                     ex[:, s_, off:],
                                     start=(t == 0), stop=(t == T - 1))
                # cover the denominator-accumulator latency before dn
                pop_c(qc)
                dn = acc_ps.tile([P, SC], F32, tag="acc")
                if dn_on_pe:
                    for t in range(T):
                        ex, s_, off = exs[t]
                        nc.tensor.matmul(dn[:, off:], ones_sb[:],
                                         ex[:, s_, off:],
                                         start=(t == 0), stop=(t == T - 1))
                else:
                    # denominator: fold the running accumulator's halves,
                    # then one ones-matmul broadcasts the per-qi sum to all
                    # 128 partitions (replaces a T-matmul chain on the PE).
                    dnf = rcp.tile([P, SC], BF16, tag="dnf")
                    nc.vector.tensor_tensor(dnf[:], dacc[:, 0, :],
                                            dacc[:, 1, :], add)
                    nc.tensor.matmul(dn[:], ones_sb[:], dnf[:],
                                     start=True, stop=True)
                rc = rcp.tile([P, SC], F32, tag="rc")
                nc.vector.reciprocal(rc[:], dn[:])
                nc.vector.tensor_tensor(
                    attnT_sb[:, h, SC * qc:SC * (qc + 1)], pv[:], rc[:], mul)
            # flush what B(qc) didn't absorb of round qc-1's C work
            if qc >= 1:
                while cwork[qc - 1]:
                    qt, mc, st = cwork[qc - 1].pop(0)
                    c_chain(qt, mc, st)()

        stage_boundary()

        if "C" not in phases and "B" in phases:
            scr2 = nc.dram_tensor("scratch2", [P, GH * S], BF16,
                                  kind="Internal")
            nc.sync.dma_start(scr2.ap(),
                              attnT_sb[:].rearrange("p h s -> p (h s)"))

        # ---- phase C tail: the last round's wo chains have no B work to
        # hide behind; emit them as a dense block over 4 psum banks ----
        for qc in range(NSC):
            while cwork[qc]:
                qt, mc, st = cwork[qc].pop(0)
                c_chain(qt, mc, st, tail=True)()

        loop_cm.__exit__(None, None, None)

    nc.compile()
    return nc


# head-dim de-interleave permutation: [0,2,...,126, 1,3,...,127]
_PERM = np.concatenate([np.arange(0, HD, 2), np.arange(1, HD, 2)])


def _prep_inputs(x, wq, wk, wv, wo, cos, sin):
    """Host-side shard + layout prep. Returns list of 8 per-core input maps."""
    bf = ml_dtypes.bfloat16
    xT = np.ascontiguousarray(x.T.astype(bf))
    # cos/sin tables duplicated across both 64-partition halves
    cosT = np.ascontiguousarray(cos.T.astype(bf))           # [64, S]
    sinT = np.ascontiguousarray(sin.T.astype(bf))
    cos2 = np.concatenate([cosT, cosT], axis=0)             # [128, S]
    sin2 = np.concatenate([sinT, sinT], axis=0)
    in_maps = []
    for c in range(NCORES):
        wq_c = wq[GH * HD * c:GH * HD * (c + 1)]            # [512, DIM]
        # de-interleave head dim within each head
        wq_cp = wq_c.reshape(GH, HD, DIM)[:, _PERM, :].reshape(GH * HD, DIM)
        wk_cp = wk[HD * c:HD * (c + 1)][_PERM, :]           # [128, DIM]
        wv_c = wv[HD * c:HD * (c + 1)]                      # [128, DIM] (no perm)
        wo_c = wo[:, GH * HD * c:GH * HD * (c + 1)]         # [DIM, 512]
        in_maps.append({
            "xt": xT,
            "wqt": np.ascontiguousarray(wq_cp.T.astype(bf)),
            "wkt": np.ascontiguousarray(wk_cp.T.astype(bf)),
            "wvt": np.ascontiguousarray(wv_c.T.astype(bf)),
            "wot": np.ascontiguousarray(wo_c.T.astype(bf)),
            "cos2": cos2,
            "sin2": sin2,
        })
    return in_maps


def kernel(x, wq, wk, wv, wo, cos, sin, window):
    assert int(window) >= S, "kernel hardcodes window >= S (plain causal)"
    x = np.asarray(x, dtype=np.float32)
    wq = np.asarray(wq, dtype=np.float32)
    wk = np.asarray(wk, dtype=np.float32)
    wv = np.asarray(wv, dtype=np.float32)
    wo = np.asarray(wo, dtype=np.float32)
    cos = np.asarray(cos, dtype=np.float32)
    sin = np.asarray(sin, dtype=np.float32)

    if "nc" not in _CACHE:
        _CACHE["nc"] = _build_bass()
    nc = _CACHE["nc"]
    in_maps = _prep_inputs(x, wq, wk, wv, wo, cos, sin)
    res = run_bass_kernel_spmd(nc, in_maps, core_ids=list(range(NCORES)))
    total = res.results[0]["out"].astype(np.float32)
    for c in range(1, NCORES):
        total += res.results[c]["out"].astype(np.float32)
    return total



# revision 13
# speedup vs baseline: 1.0092x; 1.0092x over previous
"""Trainium2 Bass kernel: GQA attention layer (RoPE + causal sliding-window)
tensor-parallel across heads on 8 NeuronCores.

Problem shapes (hardcoded): S=2048 tokens, DIM=4096, HQ=32 q-heads,
HKV=8 kv-heads, HD=128 head dim, window=2048 (window >= S, so the mask is
plain causal).

Sharding: core c owns kv-head c and q-heads 4c..4c+3 (column-parallel
wq/wk/wv, row-parallel wo). Each core computes a full [S, DIM] partial of
the output projection in f16; the host sums the 8 partials in f32.

Device-side design (v3):
 - All matmul operands bf16 (fp32 accumulate in PSUM). fp8/DoubleRow was
   measured and rejected: each fp8-quantized operand alone contributes
   2.3-4.2% l2 (softmax amplifies q/k noise; value-path errors pass
   through GEMMs undiminished) vs the 2e-2 gate.
 - Projections computed in transposed layout qT/kT [HD, S] (what the
   scoresT QK matmul wants); head dim de-interleaved via host-permuted
   wq/wk columns so RoPE is partition-aligned. sin table is sign-folded
   ([-s;s]) so RoPE = ps*cos2 + swap(ps)*sin2: 2 ACT partition-swap
   copies + 3 DVE/Pool ops per [128, 512] block.
 - Diagonal score blocks are computed full width (their upper triangle
   is valid-but-non-causal scores, so exp stays bounded) and masked
   after exp by a DVE multiply with constant 0/1 bf16 triangle tiles:
   no affine_select, no memsets, exp always processes [128, 1024]
   psum pairs. PV matmuls still skip fully-masked columns.
 - Softmax denominator: running [128, 1024] accumulate of the exp tiles
   on the Pool engine + one ones-matmul partition broadcast per
   (qc, head). No max subtraction (|scores*scale| < ~7).
 - Emission interleaves phases: A(qc+1) projection chains and C(qc-1)
   wo chains are queued as PE "filler" generators and pumped between
   B(qc) score pair-rounds, so the PE never idles on ACT exp latency
   (keeps the HAM clock-gate warm) and ACT/DVE/Pool work hides under
   PE-bound projection segments.
 - Host-side input prep pre-shuffles x/w into the exact SBUF layouts
   (per-partition rows fully contiguous) so input DMAs are dense.
"""

from collections import deque
from contextlib import ExitStack, nullcontext

import numpy as np
import ml_dtypes

import concourse.bass as bass
import concourse.mybir as mybir
import concourse.tile as tile
from concourse import bacc
from concourse.bass_utils import run_bass_kernel_spmd

S = 2048
DIM = 4096
HQ, HKV, HD = 32, 8, 128
NCORES = 8
GH = HQ // HKV          # q heads per core (= per kv head) = 4
P = 128
KT = DIM // P           # 32 contraction tiles
SC = 512                # s-chunk (psum free dim)
NSC = S // SC           # 4
NQT = S // P            # 16 query tiles of 128
NMC = DIM // SC         # 8 output column chunks
SCALE = float(HD) ** -0.5

F32 = mybir.dt.float32
F16 = mybir.dt.float16
BF16 = mybir.dt.bfloat16

_CACHE = {}


def _build_bass(loop_n=1, staggered=False, phases="ABC", dn_on_pe=False):
    """loop_n > 1 wraps the whole body in a hardware For_i loop — used only
    by the test harness for differential wall-clock timing (the axon
    dispatch floor is ~45-85 ms, far above the kernel's execution time)."""
    nc = bacc.Bacc("TRN2", target_bir_lowering=False, debug=False,
                   enable_asserts=False)
    # host pre-shuffled layouts (see _prep_inputs): per-partition rows are
    # fully contiguous so every input DMA is dense.
    x_d = nc.dram_tensor("xs", [P, NSC, KT, SC], BF16, kind="ExternalInput")
    wq_d = nc.dram_tensor("wqs", [P, KT, GH * HD], BF16, kind="ExternalInput")
    wk_d = nc.dram_tensor("wks", [P, KT, HD], BF16, kind="ExternalInput")
    wv_d = nc.dram_tensor("wvs", [P, KT, HD], BF16, kind="ExternalInput")
    wo_d = nc.dram_tensor("wos", [P, GH, DIM], BF16, kind="ExternalInput")
    cos_d = nc.dram_tensor("cos2", [P, S], BF16, kind="ExternalInput")
    sin_d = nc.dram_tensor("sin2", [P, S], BF16, kind="ExternalInput")
    msk_d = nc.dram_tensor("dmask", [P, 896], BF16, kind="ExternalInput")
    out_d = nc.dram_tensor("out", [S, DIM], F16, kind="ExternalOutput")

    with tile.TileContext(nc) as tc, ExitStack() as ctx:
        consts = ctx.enter_context(tc.tile_pool(name="consts", bufs=1))
        state = ctx.enter_context(tc.tile_pool(name="state", bufs=1))
        xpool = ctx.enter_context(tc.tile_pool(name="xpool", bufs=2))
        ropep = ctx.enter_context(tc.tile_pool(name="ropep", bufs=1))
        cspool = ctx.enter_context(tc.tile_pool(name="cspool", bufs=2))
        expp = ctx.enter_context(tc.tile_pool(name="expp", bufs=6))
        osb = ctx.enter_context(tc.tile_pool(name="osb", bufs=2))
        attp = ctx.enter_context(tc.tile_pool(name="attp", bufs=2))
        rcp = ctx.enter_context(tc.tile_pool(name="rcp", bufs=2))
        vtp = ctx.enter_context(tc.tile_pool(name="vtp", bufs=1))
        # PSUM (8 banks): mm_ps 2 (A + C chains), sc_ps 2x2 (score pairs),
        # acc_ps 2 (pv + dn).
        mm_ps = ctx.enter_context(tc.tile_pool(name="mm_ps", bufs=2, space="PSUM"))
        sc_ps = ctx.enter_context(tc.tile_pool(name="sc_ps", bufs=2, space="PSUM"))
        acc_ps = ctx.enter_context(tc.tile_pool(name="acc_ps", bufs=2, space="PSUM"))

        ALL_ENGINES = (mybir.EngineType.PE, mybir.EngineType.DVE,
                       mybir.EngineType.Activation, mybir.EngineType.Pool,
                       mybir.EngineType.SP)
        loop_cm = (tc.For_i(0, loop_n, 1, staggered_reset=staggered,
                            hint_engines=ALL_ENGINES)
                   if loop_n > 1 else nullcontext())
        loop_cm.__enter__()

        # ---- constants / weights in SBUF ----
        # wk first on gpsimd (the k chain runs first and needs it); xc0
        # round-robins over the three DMA queues; wq splits sync+scalar.
        wk_sb = consts.tile([P, KT, HD], BF16)
        nc.gpsimd.dma_start(wk_sb[:], wk_d.ap())
        xc0 = xpool.tile([P, KT, SC], BF16, tag="x")
        x_engs = [nc.sync, nc.scalar, nc.gpsimd]
        for og in range(4):
            x_engs[og % 3].dma_start(
                xc0[:, 8 * og:8 * og + 8, :], x_d.ap()[:, 0, 8 * og:8 * og + 8, :])
        wv_sb = consts.tile([P, KT, HD], BF16)
        nc.gpsimd.dma_start(wv_sb[:], wv_d.ap())
        msk_sb = consts.tile([P, 896], BF16)
        nc.gpsimd.dma_start(msk_sb[:], msk_d.ap())
        wq_sb = consts.tile([P, KT, GH * HD], BF16)
        for og in range(4):
            eng = nc.sync if og % 2 == 1 else nc.scalar
            eng.dma_start(wq_sb[:, 8 * og:8 * og + 8, :],
                          wq_d.ap()[:, 8 * og:8 * og + 8, :])
        ones_sb = consts.tile([P, P], BF16)
        nc.vector.memset(ones_sb[:], 1.0)

        # state tiles
        qT_sb = state.tile([P, GH, S], BF16)     # rope'd q, permuted head dim
        kT_sb = state.tile([P, S], BF16)         # rope'd k, permuted head dim
        v_sb = state.tile([P, NQT, HD], BF16)    # v natural [s-tile, d]

        H = 64
        mul = mybir.AluOpType.mult
        add = mybir.AluOpType.add
        CP = mybir.ActivationFunctionType.Copy
        EXP = mybir.ActivationFunctionType.Exp

        def rope(ps, out_sl, sc):
            """ps: [128, 512] f32 psum, head dim de-interleaved (even dims
            at partitions 0..63, odd at 64..127). Writes bf16 out_sl.
            sin2's top half is negated, so out = ps*cos2 + swap(ps)*sin2."""
            cs, sn = cs_tiles[sc]
            Asw = ropep.tile([P, SC], BF16, tag="ropeA")
            P1 = ropep.tile([P, SC], BF16, tag="ropeB")
            nc.scalar.activation(Asw[0:H], ps[H:P], CP)
            nc.scalar.activation(Asw[H:P], ps[0:H], CP)
            nc.vector.tensor_tensor(P1[:], ps[:], cs, mul)
            nc.gpsimd.tensor_tensor(Asw[:], Asw[:], sn, mul)
            nc.vector.tensor_tensor(out_sl, P1[:], Asw[:], add)

        # ---- phase A generators: QKV projections + RoPE per s-chunk ----
        def mm_chain(ps, w_sb, xc, col0, ncol):
            """32-step bf16 accumulation chain; yields every 8."""
            for o in range(KT):
                nc.tensor.matmul(ps[:], w_sb[:, o, col0:col0 + ncol],
                                 xc[:, o, :], start=(o == 0),
                                 stop=(o == KT - 1))
                if o % 8 == 7 and o != KT - 1:
                    yield

        def a_chunk_gen(sc):
            cs = cspool.tile([P, SC], BF16, tag="cos", name=f"cos{sc}")
            sn = cspool.tile([P, SC], BF16, tag="sin", name=f"sin{sc}")
            nc.scalar.dma_start(cs[:], cos_d.ap()[:, SC * sc:SC * (sc + 1)])
            nc.scalar.dma_start(sn[:], sin_d.ap()[:, SC * sc:SC * (sc + 1)])
            cs_tiles[sc] = (cs, sn)
            if sc < NSC - 1:
                # prefetch next chunk's x while this one computes
                xn = xpool.tile([P, KT, SC], BF16, tag="x", name=f"x{sc + 1}")
                for og in range(4):
                    nc.sync.dma_start(xn[:, 8 * og:8 * og + 8, :],
                                      x_d.ap()[:, sc + 1, 8 * og:8 * og + 8, :])
                a_chunk_gen.xn[sc + 1] = xn
            xc = a_chunk_gen.xn[sc]
            # k chain
            ps = mm_ps.tile([P, SC], F32, tag="mm", name=f"kps{sc}")
            yield from mm_chain(ps, wk_sb, xc, 0, HD)
            yield
            rope(ps, kT_sb[:, SC * sc:SC * (sc + 1)], sc)
            # v chain
            ps = mm_ps.tile([P, SC], F32, tag="mm", name=f"vps{sc}")
            yield from mm_chain(ps, wv_sb, xc, 0, HD)
            yield
            vt = vtp.tile([P, SC], BF16, tag="vt")
            nc.scalar.activation(vt[:], ps[:], CP)
            for b in range(4):
                nc.sync.dma_start_transpose(v_sb[:, 4 * sc + b, :],
                                            vt[:, P * b:P * (b + 1)])
            # q chains
            for h in range(GH):
                ps = mm_ps.tile([P, SC], F32, tag="mm", name=f"qps{sc}_{h}")
                yield from mm_chain(ps, wq_sb, xc, HD * h, HD)
                yield
                rope(ps, qT_sb[:, h, SC * sc:SC * (sc + 1)], sc)
        a_chunk_gen.xn = {0: xc0}
        cs_tiles = {}

        # wo weights ride gpsimd after wk/wv/x0/masks; needed from C on
        wo_sb = consts.tile([P, GH, DIM], BF16)
        nc.gpsimd.dma_start(wo_sb[:], wo_d.ap())

        # ---- phase C generators: wo chains ----
        def c_chain_gen(qt):
            attnT = at_tiles[qt // 4]
            ostg = None
            for mc in range(NMC):
                if mc % 2 == 0:
                    ostg = osb.tile([P, 2 * SC], F16, tag="ob",
                                    name=f"ostg_{qt}_{mc}")
                wps = mm_ps.tile([P, SC], F32, tag="mm", name=f"wps{qt}_{mc}")
                for h in range(GH):
                    nc.tensor.matmul(wps[:],
                                     attnT[:, h, P * (qt % 4):P * (qt % 4 + 1)],
                                     wo_sb[:, h, SC * mc:SC * (mc + 1)],
                                     start=(h == 0), stop=(h == GH - 1))
                dst = ostg[:, SC * (mc % 2):SC * (mc % 2 + 1)]
                # GPSIMD cannot read PSUM on HW: alternate DVE/ACT only
                if mc % 2 == 0:
                    nc.vector.tensor_copy(dst, wps[:])
                else:
                    nc.scalar.activation(dst, wps[:], CP)
                if mc % 2 == 1:
                    nc.sync.dma_start(
                        out_d.ap()[P * qt:P * (qt + 1),
                                   SC * (mc - 1):SC * (mc + 1)], ostg[:])
                yield

        # ---- filler scheduler ----
        a_fill = deque()
        c_fill = deque()
        at_tiles = {}

        def pump(n):
            for _ in range(n):
                q = a_fill if a_fill else c_fill
                if not q:
                    return
                try:
                    next(q[0])
                except StopIteration:
                    q.popleft()

        def drain(q):
            while q:
                try:
                    next(q[0])
                except StopIteration:
                    q.popleft()

        # ---- main schedule ----
        # A(0) runs alone (nothing to interleave yet)
        drain(deque([a_chunk_gen(0)]))

        for qc in range(NSC):
            T = 4 * qc + 4
            JP = T // 2
            if qc < NSC - 1:
                a_fill.append(a_chunk_gen(qc + 1))
            if qc >= 1:
                for qt in range(4 * (qc - 1), 4 * qc):
                    c_fill.append(c_chain_gen(qt))
            at_tiles[qc] = attp.tile([P, GH, SC], BF16, tag="at",
                                     name=f"attnT{qc}")
            for h in range(GH):
                q_sl = qT_sb[:, h, SC * qc:SC * (qc + 1)]
                exs = []
                dacc = None
                for j in range(JP):
                    sps = sc_ps.tile([P, 2, SC], F32, tag="sc")
                    ex = expp.tile([P, 2, SC], BF16, tag="exp")
                    for s_ in range(2):
                        t = 2 * j + s_
                        nc.tensor.matmul(sps[:, s_, :],
                                         kT_sb[:, P * t:P * (t + 1)],
                                         q_sl, start=True, stop=True)
                    nc.scalar.activation(ex[:], sps[:], EXP, scale=SCALE)
                    if j >= 2 * qc:
                        # diagonal pair: zero the non-causal upper triangle.
                        # msk_sb[p, y] = 1 iff y >= p + 384, so the slice at
                        # offset 384-128*d is the d-th triangle mask.
                        for s_ in range(2):
                            off = 384 - P * (2 * (j - 2 * qc) + s_)
                            nc.vector.tensor_tensor(
                                ex[:, s_, :], ex[:, s_, :],
                                msk_sb[:, off:off + SC], mul)
                    exs.append(ex)
                    if j == 1:
                        dacc = expp.tile([P, 2, SC], BF16, tag="dacc",
                                         bufs=1)
                        nc.vector.tensor_tensor(dacc[:], exs[0][:],
                                                exs[1][:], add)
                    elif j >= 2:
                        nc.vector.tensor_tensor(dacc[:], dacc[:], ex[:],
                                                add)
                    pump(1)
                dnf = rcp.tile([P, SC], BF16, tag="dnf", bufs=1)
                nc.vector.tensor_tensor(dnf[:], dacc[:, 0, :],
                                        dacc[:, 1, :], add)
                # PV chain (skip fully-masked columns on diagonal tiles)
                pv = acc_ps.tile([P, SC], F32, tag="acc")
                for t in range(T):
                    off = max(0, P * (t - 4 * qc))
                    nc.tensor.matmul(pv[:, off:], v_sb[:, t, :],
                                     exs[t // 2][:, t % 2, off:],
                                     start=(t == 0), stop=(t == T - 1))
                pump(1)
                dn = acc_ps.tile([P, SC], F32, tag="acc")
                nc.tensor.matmul(dn[:], ones_sb[:], dnf[:],
                                 start=True, stop=True)
                rc = rcp.tile([P, SC], F32, tag="rc", bufs=1)
                nc.vector.reciprocal(rc[:], dn[:])
                nc.vector.tensor_tensor(
                    at_tiles[qc][:, h, :], pv[:], rc[:], mul)
                pump(2)
            # A(qc+1) must be fully emitted before scores(qc+1) start;
            # C(qc-1) must drain so the 2-deep attnT rotation stays safe
            drain(a_fill)
            drain(c_fill)

        for qt in range(4 * (NSC - 1), NQT):
            c_fill.append(c_chain_gen(qt))
        drain(c_fill)

        loop_cm.__exit__(None, None, None)

    nc.compile()
    return nc


# head-dim de-interleave permutation: [0,2,...,126, 1,3,...,127]
_PERM = np.concatenate([np.arange(0, HD, 2), np.arange(1, HD, 2)])


def _prep_inputs(x, wq, wk, wv, wo, cos, sin):
    """Host-side shard + layout prep. Returns list of 8 per-core input maps.

    All tensors are pre-shuffled to the exact SBUF layouts (partition dim
    first, per-partition rows contiguous) so device DMAs are dense."""
    bf = ml_dtypes.bfloat16
    # x: [S, DIM] -> [P, NSC, KT, SC] where element (p, sc, o, s) =
    # x[sc*SC + s, o*P + p]
    xs = np.ascontiguousarray(
        x.T.astype(bf).reshape(KT, P, NSC, SC).transpose(1, 2, 0, 3))
    # cos/sin tables duplicated across both halves; sin top half negated
    cosT = np.ascontiguousarray(cos.T.astype(np.float32))         # [64, S]
    sinT = np.ascontiguousarray(sin.T.astype(np.float32))
    cos2 = np.concatenate([cosT, cosT], axis=0).astype(bf)        # [128, S]
    sin2 = np.concatenate([-sinT, sinT], axis=0).astype(bf)
    # extended diagonal mask: dmask[p, y] = 1 iff y >= p + 384; the
    # [*, 384-128*d : 896-128*d] slice is the d-th triangle mask
    pidx = np.arange(P)[:, None]
    yidx = np.arange(896)[None, :]
    dmask = (yidx >= pidx + 384).astype(bf)

    def wshuf(w):       # [DIM, M] -> [P, KT, M]
        m = w.shape[1]
        return np.ascontiguousarray(w.reshape(KT, P, m).transpose(1, 0, 2))

    in_maps = []
    for c in range(NCORES):
        wq_c = wq[GH * HD * c:GH * HD * (c + 1)]            # [512, DIM]
        # de-interleave head dim within each head
        wq_cp = wq_c.reshape(GH, HD, DIM)[:, _PERM, :].reshape(GH * HD, DIM)
        wk_cp = wk[HD * c:HD * (c + 1)][_PERM, :]           # [128, DIM]
        wv_c = wv[HD * c:HD * (c + 1)]                      # [128, DIM]
        wo_c = wo[:, GH * HD * c:GH * HD * (c + 1)]         # [DIM, 512]
        wos = np.ascontiguousarray(
            wo_c.T.astype(bf).reshape(GH, P, DIM).transpose(1, 0, 2))
        in_maps.append({
            "xs": xs,
            "wqs": wshuf(wq_cp.T.astype(bf)),
            "wks": wshuf(wk_cp.T.astype(bf)),
            "wvs": wshuf(wv_c.T.astype(bf)),
            "wos": wos,
            "cos2": cos2,
            "sin2": sin2,
            "dmask": dmask,
        })
    return in_maps


def kernel(x, wq, wk, wv, wo, cos, sin, window):
    assert int(window) >= S, "kernel hardcodes window >= S (plain causal)"
    x = np.asarray(x, dtype=np.float32)
    wq = np.asarray(wq, dtype=np.float32)
    wk = np.asarray(wk, dtype=np.float32)
    wv = np.asarray(wv, dtype=np.float32)
    wo = np.asarray(wo, dtype=np.float32)
    cos = np.asarray(cos, dtype=np.float32)
    sin = np.asarray(sin, dtype=np.float32)

    if "nc" not in _CACHE:
        _CACHE["nc"] = _build_bass()
    nc = _CACHE["nc"]
    in_maps = _prep_inputs(x, wq, wk, wv, wo, cos, sin)
    res = run_bass_kernel_spmd(nc, in_maps, core_ids=list(range(NCORES)))
    total = res.results[0]["out"].astype(np.float32)
    for c in range(1, NCORES):
        total += res.results[c]["out"].astype(np.float32)
    return total
